# revision 1
# baseline (speedup 1.0000x reference)
"""Multi-head attention Trainium2 kernel (8 NeuronCores, SPMD).

Problem: B=2, S=2048, D=1024, H=16 heads, DK=DV=64.
Sharding: batch (2) x head-groups (4 heads per core) = 8 cores.
Each core computes, for its batch b and its 4 heads:
    Q/K/V projections, attention, and the partial output projection
    out_partial = concat_heads(ctx) @ Wo[head rows] + bo/4.
Host gathers by summing the 4 partials per batch (row-parallel TP reduce).

Kernel dataflow highlights:
  - All big matmuls run as float32r (full PE rate for moving dim >= 256).
  - Scores are computed TRANSPOSED (S^T = K Q^T) so the exp() evacuation
    directly yields P^T in the [t, s] layout the ctx matmul consumes.
  - A ones-column appended to V makes the softmax denominator fall out of
    the ctx matmul (row 64 of the 65-row PSUM accumulator); normalization
    is a cheap reciprocal + partition-broadcast + multiply.
  - No max-subtraction: scores are ~N(0, 0.33), exp cannot overflow, and
    softmax is shift-invariant so the result matches the reference.
"""
import sys

if "/opt/trn_rl_repo" not in sys.path:
    sys.path.insert(0, "/opt/trn_rl_repo")

import numpy as np

import bass_rust
import concourse.bass as bass
import concourse.mybir as mybir
import concourse.tile as tile
from concourse.bass_utils import run_bass_kernel_spmd
from concourse.masks import make_identity
from concourse.vector_clock import ScopedClock

F32 = mybir.dt.float32
F32R = mybir.dt.float32r
BF16 = mybir.dt.bfloat16
AF = mybir.ActivationFunctionType

B, S, D = 2, 2048, 1024
H, DK, DV = 16, 64, 64
HL = 4          # heads per core
NPAIR = 2       # head pairs per core (2 heads packed per 128 partitions)
ST = S // 128   # 16 s-tiles (and t-tiles)
DT = D // 128   # 8 d-tiles
SC = 1024       # attention s-chunk (psum free size)
NSC = S // SC   # 2
N_CORES = 8


class _TileContextSplitDrain(tile.TileContext):
    """Walrus in this container rejects ANY instruction carrying >1 sem wait
    ("Too many sync wait commands"). Post-lowering, sweep every basic block
    and move surplus waits onto injected EventSemaphore carrier instructions
    placed immediately before the over-subscribed instruction (same engine,
    same program point - semantics unchanged). Also emit the final drain as
    one drain per logical proc so each carries a single wait."""

    _MAXW = 1

    def _split_excess_waits(self):
        nc = self.nc
        for fn in nc.m.functions:
            for bb in fn.blocks:
                insts = bb.instructions
                new_list = []
                changed = False
                for ins in insts:
                    si = ins.sync_info
                    waits = list(si.on_wait) if si is not None and si.on_wait else []
                    if len(waits) > self._MAXW:
                        changed = True
                        extra, keep = waits[:-self._MAXW], waits[-self._MAXW:]
                        for k in range(0, len(extra), self._MAXW):
                            chunk = extra[k:k + self._MAXW]
                            ev = mybir.InstEventSemaphore(
                                name=f"wsplit_{nc.next_id()}", ins=[], outs=[]
                            )
                            ev.engine = ins.engine
                            ev.sync_info = bass_rust.SyncInfo(
                                on_wait=chunk, on_update=[]
                            )
                            nc.register_instruction(ev, overwrite=True)
                            new_list.append(ev)
                        ins.sync_info = bass_rust.SyncInfo(
                            on_wait=keep,
                            on_update=list(si.on_update) if si.on_update else [],
                        )
                    new_list.append(ins)
                if changed:
                    insts[:] = new_list

    def _drain_and_barrier(self, tick_clock, wait_clock):
        self._split_excess_waits()
        ticks = list(tick_clock.global_clock)
        for p, t in enumerate(ticks):
            if t <= 0:
                continue
            v = bass_rust.VectorClock()
            v.require_at_least(p, t)
            d = self.nc.sync.drain()
            wait_clock.add_sem_waits(d.ins, ScopedClock({None: v}))
        self.nc.all_engine_barrier()
        popped = self.nc._tile_sem_poison_stack.pop()
        assert popped is self._sem_poison
        self.nc.clear_and_free_semaphores(list(self.sems.allocated().values()))
        self.nc.all_engine_barrier()


def build_nc(debug: bool = False) -> bass.Bass:
    nc = bass.Bass()

    x_d = nc.dram_tensor("x", [S, D], F32, kind="ExternalInput")
    wqkv_d = nc.dram_tensor("wqkv", [D, 3 * HL * DK], F32R, kind="ExternalInput")
    bqkv_d = nc.dram_tensor("bqkv", [128, 6], F32, kind="ExternalInput")
    wo_d = nc.dram_tensor("wo", [HL * DV, D], F32R, kind="ExternalInput")
    bo4_d = nc.dram_tensor("bo4", [1, D], F32, kind="ExternalInput")
    out_d = nc.dram_tensor("out", [S, D], F32, kind="ExternalOutput")
    dbg = {}
    if debug:
        dbg["xT"] = nc.dram_tensor("dbg_xT", [128, DT, S], F32R, kind="ExternalOutput")
        dbg["QT"] = nc.dram_tensor("dbg_QT", [128, NPAIR, S], F32R, kind="ExternalOutput")
        dbg["KT"] = nc.dram_tensor("dbg_KT", [128, NPAIR, S], F32R, kind="ExternalOutput")
        dbg["VA"] = nc.dram_tensor("dbg_VA", [128, ST, HL, 66], mybir.dt.bfloat16, kind="ExternalOutput")
        dbg["ctxraw"] = nc.dram_tensor("dbg_ctxraw", [128, NPAIR, S], F32R, kind="ExternalOutput")
        dbg["rs"] = nc.dram_tensor("dbg_rs", [1, HL, NSC, SC], F32, kind="ExternalOutput")
        dbg["ctxn"] = nc.dram_tensor("dbg_ctxn", [128, NPAIR, S], F32R, kind="ExternalOutput")

    with _TileContextSplitDrain(nc) as tc:
        with (
            tc.tile_pool(name="const", bufs=1) as constp,
            tc.tile_pool(name="persist", bufs=1) as pers,
        ):
            identity = constp.tile([128, 128], F32, tag="identity")
            make_identity(nc, identity)
            bqkv_sb = constp.tile([128, 6], F32, tag="bqkv")
            nc.sync.dma_start(out=bqkv_sb, in_=bqkv_d[:, :])
            bo_sb = constp.tile([1, D], F32, tag="bo")
            nc.sync.dma_start(out=bo_sb, in_=bo4_d[:, :])
            bo_rep = constp.tile([128, D], F32, tag="bo_rep")
            nc.sync.dma_start(out=bo_rep, in_=bo4_d[0:1, :].to_broadcast((128, D)))
            wo_sb = constp.tile([128, 2, D], F32R, tag="wo")
            for p in range(2):
                nc.sync.dma_start(out=wo_sb[:, p, :], in_=wo_d[p * 128:(p + 1) * 128, :])

            # Persistent activation-side tensors
            QT = pers.tile([128, NPAIR, S], F32R, tag="QT")
            KT = pers.tile([128, NPAIR, S], F32R, tag="KT")
            ctxT = pers.tile([128, NPAIR, S], F32R, tag="ctxT")
            V_aug = pers.tile([128, ST, HL, 66], BF16, tag="V_aug")

            # ---------------- Phase 1+2+3: x load/transpose, QKV, V_aug -----
            with (
                tc.tile_pool(name="xtiles", bufs=3) as xp,
                tc.tile_pool(name="wtiles", bufs=8) as wp,
                tc.tile_pool(name="xT", bufs=1) as xtp,
                tc.tile_pool(name="VT", bufs=1) as vtp,
            ):
                xT = xtp.tile([128, DT, S], F32R, tag="xT")
                VT = vtp.tile([128, NPAIR, S], F32, tag="VT")

                w_sb = []
                for d in range(DT):
                    wt = wp.tile([128, 3 * HL * DK], F32R, tag="w")
                    nc.sync.dma_start(out=wt, in_=wqkv_d[d * 128:(d + 1) * 128, :])
                    w_sb.append(wt)

                # P1: transpose x into xT[d, s]
                with tc.tile_pool(name="trps", bufs=2, space="PSUM") as trp:
                    for i in range(ST):
                        xt = xp.tile([128, D], F32, tag="x")
                        nc.sync.dma_start(out=xt, in_=x_d[i * 128:(i + 1) * 128, :])
                        for jg in range(2):
                            ps = trp.tile([128, 512], F32, tag="trps")
                            for k in range(4):
                                j = jg * 4 + k
                                nc.tensor.transpose(
                                    ps[:, k * 128:(k + 1) * 128],
                                    xt[:, j * 128:(j + 1) * 128],
                                    identity,
                                )
                            nc.vector.tensor_copy(
                                xT[:, jg * 4:(jg + 1) * 4, i * 128:(i + 1) * 128],
                                ps.rearrange("p (a b) -> p a b", a=4),
                            )

                # P2: Q^T / K^T / V^T projections (pair-packed partitions)
                with tc.tile_pool(name="qkvps", bufs=8, space="PSUM") as qp:
                    for proj, dest in ((0, QT), (1, KT), (2, VT)):
                        for pair in range(NPAIR):
                            pss = [qp.tile([128, 512], F32, name="qkvps", tag="qkvps") for _ in range(4)]
                            for d in range(DT):
                                lhs = w_sb[d][:, proj * 256 + pair * 128: proj * 256 + (pair + 1) * 128]
                                for c4 in range(4):
                                    nc.tensor.matmul(
                                        pss[c4][:, :],
                                        lhs,
                                        xT[:, d, c4 * 512:(c4 + 1) * 512],
                                        start=(d == 0),
                                        stop=(d == DT - 1),
                                    )
                            bias_ap = bqkv_sb[:, proj * 2 + pair: proj * 2 + pair + 1]
                            for c4 in range(4):
                                nc.vector.tensor_scalar_add(
                                    dest[:, pair, c4 * 512:(c4 + 1) * 512],
                                    pss[c4][:, :],
                                    bias_ap,
                                )

                if debug:
                    nc.sync.dma_start(out=dbg["xT"][:, :, :], in_=xT[:, :, :])

                # P3: V_aug[t, j, h, 0:64] = V blocks (bf16), col 64 = ones
                nc.vector.memset(V_aug[:, :, :, 64:65], 1.0)
                nc.vector.memset(V_aug[:, :, :, 65:66], 0.0)
                with tc.tile_pool(name="vps", bufs=2, space="PSUM") as vp:
                    for pair in range(NPAIR):
                        for j in range(ST):
                            ps = vp.tile([128, 128], F32, tag="vps")
                            nc.tensor.transpose(
                                ps, VT[:, pair, j * 128:(j + 1) * 128], identity
                            )
                            nc.vector.tensor_copy(
                                V_aug[:, j, 2 * pair:2 * pair + 2, 0:64],
                                ps.rearrange("p (h v) -> p h v", h=2),
                            )

            # ---------------- Phase 4+5: attention + normalize --------------
            with (
                tc.tile_pool(name="ptp", bufs=3) as ptp,
                tc.tile_pool(name="rsp", bufs=1) as rsp,
                tc.tile_pool(name="repp", bufs=1) as repp,
                tc.tile_pool(name="dramsc", bufs=1, space="DRAM") as drp,
                tc.tile_pool(name="stps", bufs=3, space="PSUM") as stp,
                tc.tile_pool(name="ctxps", bufs=1, space="PSUM") as cxp,
            ):
                rs = rsp.tile([1, HL, NSC, SC], F32, tag="rs")
                for pair in range(NPAIR):
                    for e in range(2):
                        h = 2 * pair + e
                        for c in range(NSC):
                            cp = cxp.tile([128, SC], F32, tag="cp")
                            sps = {}

                            def emit_scores(jj, _e=e, _pair=pair, _c=c, _sps=None):
                                sp = stp.tile([128, SC], F32, name="sp", tag="sp")
                                lhs = KT[64 * _e:64 * (_e + 1), _pair, jj * 128:(jj + 1) * 128]
                                for half in range(2):
                                    nc.tensor.matmul(
                                        sp[:, half * 512:(half + 1) * 512],
                                        lhs,
                                        QT[64 * _e:64 * (_e + 1), _pair,
                                           _c * SC + half * 512: _c * SC + (half + 1) * 512],
                                        start=True,
                                        stop=True,
                                    )
                                sps[jj] = sp

                            emit_scores(0)
                            emit_scores(1)
                            for j in range(ST):
                                if j + 2 < ST:
                                    emit_scores(j + 2)
                                sp = sps.pop(j)
                                pt = ptp.tile([128, SC], BF16, tag="pt")
                                nc.scalar.activation(out=pt, in_=sp, func=AF.Exp)
                                for half in range(2):
                                    nc.tensor.matmul(
                                        cp[0:65, half * 512:(half + 1) * 512],
                                        V_aug[:, j, h, 0:65],
                                        pt[:, half * 512:(half + 1) * 512],
                                        start=(j == 0),
                                        stop=(j == ST - 1),
                                    )
                            nc.vector.tensor_copy(rs[0:1, h, c, :], cp[64:65, :])
                            nc.vector.tensor_copy(
                                ctxT[64 * e:64 * (e + 1), pair, c * SC:(c + 1) * SC],
                                cp[0:64, :],
                            )

                if debug:
                    nc.sync.dma_start(out=dbg["QT"][:, :, :], in_=QT[:, :, :])
                    nc.sync.dma_start(out=dbg["KT"][:, :, :], in_=KT[:, :, :])
                    nc.sync.dma_start(out=dbg["VA"][:, :, :, :], in_=V_aug[:, :, :, :])
                    nc.sync.dma_start(out=dbg["ctxraw"][:, :, :], in_=ctxT[:, :, :])
                    nc.sync.dma_start(out=dbg["rs"][:, :, :, :], in_=rs[:, :, :, :])

                # P5: normalize ctxT rows by softmax denominators
                nc.vector.reciprocal(rs[0:1, :, :, :], rs[0:1, :, :, :])
                rs_d = drp.tile([HL * NSC, SC], F32, tag="rs_d")
                nc.sync.dma_start(out=rs_d[:, :], in_=rs[0:1, :, :, :])
                rep = repp.tile([128, NPAIR, S], F32, tag="rep")
                for pair in range(NPAIR):
                    for e in range(2):
                        h = 2 * pair + e
                        for c in range(NSC):
                            nc.sync.dma_start(
                                out=rep[64 * e:64 * (e + 1), pair, c * SC:(c + 1) * SC],
                                in_=rs_d[h * NSC + c, :].unsqueeze(0).to_broadcast((64, SC)),
                            )
                for pair in range(NPAIR):
                    nc.vector.tensor_mul(
                        ctxT[:, pair, :], ctxT[:, pair, :], rep[:, pair, :]
                    )

            if debug:
                nc.sync.dma_start(out=dbg["ctxn"][:, :, :], in_=ctxT[:, :, :])

            # ---------------- Phase 6: output projection --------------------
            with (
                tc.tile_pool(name="outp", bufs=3) as op,
                tc.tile_pool(name="outps", bufs=4, space="PSUM") as ops,
            ):
                for i in range(ST):
                    ot = op.tile([128, D], F32, tag="ot")
                    pss = [ops.tile([128, 512], F32, name="ops", tag="ops") for _ in range(2)]
                    for pair in range(NPAIR):
                        for dc in range(2):
                            nc.tensor.matmul(
                                pss[dc][:, :],
                                ctxT[:, pair, i * 128:(i + 1) * 128],
                                wo_sb[:, pair, dc * 512:(dc + 1) * 512],
                                start=(pair == 0),
                                stop=(pair == NPAIR - 1),
                            )
                    for dc in range(2):
                        nc.vector.tensor_add(
                            ot[:, dc * 512:(dc + 1) * 512],
                            pss[dc][:, :],
                            bo_rep[:, dc * 512:(dc + 1) * 512],
                        )
                    nc.sync.dma_start(out=out_d[i * 128:(i + 1) * 128, :], in_=ot)

    return nc


_NC_CACHE = None


def get_nc() -> bass.Bass:
    global _NC_CACHE
    if _NC_CACHE is None:
        _NC_CACHE = build_nc()
    return _NC_CACHE


def prep_in_maps(hidden_state, Wq, bq, Wk, bk, Wv, bv, Wo, bo):
    hidden_state = np.asarray(hidden_state, np.float32)
    Wq, bq = np.asarray(Wq, np.float32), np.asarray(bq, np.float32)
    Wk, bk = np.asarray(Wk, np.float32), np.asarray(bk, np.float32)
    Wv, bv = np.asarray(Wv, np.float32), np.asarray(bv, np.float32)
    Wo, bo = np.asarray(Wo, np.float32), np.asarray(bo, np.float32)
    scale = 1.0 / np.sqrt(DK).astype(np.float32)

    in_maps = []
    for c in range(N_CORES):
        b, g = c // 4, c % 4
        hs = slice(HL * g, HL * (g + 1))
        # [4, D, DK] -> [D, 4*DK] head-major columns
        wq = Wq[hs].transpose(1, 0, 2).reshape(D, HL * DK) * scale
        wk = Wk[hs].transpose(1, 0, 2).reshape(D, HL * DK)
        wv = Wv[hs].transpose(1, 0, 2).reshape(D, HL * DV)
        wqkv = np.ascontiguousarray(
            np.concatenate([wq, wk, wv], axis=1), dtype=np.float32
        )
        bq_p = (bq[hs] * scale).reshape(NPAIR, 128)
        bk_p = bk[hs].reshape(NPAIR, 128)
        bv_p = bv[hs].reshape(NPAIR, 128)
        bqkv = np.stack(
            [bq_p[0], bq_p[1], bk_p[0], bk_p[1], bv_p[0], bv_p[1]], axis=1
        ).astype(np.float32)
        in_maps.append({
            "x": np.ascontiguousarray(hidden_state[b]),
            "wqkv": wqkv,
            "bqkv": np.ascontiguousarray(bqkv),
            "wo": np.ascontiguousarray(Wo[HL * DV * g: HL * DV * (g + 1)]),
            "bo4": np.ascontiguousarray((bo / 4.0)[None, :]),
        })
    return in_maps


def gather(results):
    """Sum the 4 row-parallel partials per batch."""
    out = np.empty((B, S, D), np.float32)
    for b in range(B):
        acc = results[4 * b]["out"].astype(np.float32)
        for g in range(1, 4):
            acc = acc + results[4 * b + g]["out"]
        out[b] = acc
    return out


def kernel(**inputs) -> np.ndarray:
    nc = get_nc()
    in_maps = prep_in_maps(**inputs)
    res = run_bass_kernel_spmd(nc, in_maps, core_ids=list(range(N_CORES)))
    return gather(res.results)



# revision 6
# speedup vs baseline: 1.3682x; 1.3682x over previous
"""Multi-head attention Trainium2 kernel (8 NeuronCores, SPMD).

Problem: B=2, S=2048, D=1024, H=16 heads, DK=DV=64.
Sharding: batch (2) x head-groups (4 heads per core) = 8 cores.
Each core computes, for its batch b and its 4 heads:
    Q/K/V projections, attention, and the partial output projection
    out_partial = concat_heads(ctx) @ Wo[head rows] + bo/4.
Host gathers by summing the 4 partials per batch (row-parallel TP reduce).

v2 design notes (vs the 405us baseline):
  - x arrives pre-transposed and in bf16 from host prep: no PE transposes,
    half the input DMA bytes.
  - All matmuls run bf16 stationary x bf16 moving (full PE rate), fp32 PSUM.
  - V is computed directly in [t, (h,v)] layout (stationary = xT tile,
    moving = Wv), so no V transposes either.
  - V_aug has 64 ones-columns: the ctx matmul then yields the softmax
    denominator replicated on PSUM partitions 64..127, so normalization is
    one reciprocal_approx_fast + one fused multiply-evacuate on DVE.
    (bv is folded in AFTER normalization algebraically: adding bv to V
    would add bv*den to raw ctx; instead V carries bv and the identity
    (ctx_raw + bv*den)/den = ctx/den + bv makes the single fused multiply
    correct with bv added to V before augmentation.)
  - Chunk-outer attention loop: after each 512-wide s-chunk finishes all
    4 heads, its output projection + DMA stream out under the next chunk's
    attention, eliminating the serialized tail.
"""
import sys

if "/opt/trn_rl_repo" not in sys.path:
    sys.path.insert(0, "/opt/trn_rl_repo")

import ml_dtypes
import numpy as np

import bass_rust
import concourse.bass as bass
import concourse.mybir as mybir
import concourse.tile as tile
from concourse.bass_utils import run_bass_kernel_spmd
from concourse.vector_clock import ScopedClock

F32 = mybir.dt.float32
BF16 = mybir.dt.bfloat16
AF = mybir.ActivationFunctionType
BF16NP = ml_dtypes.bfloat16

B, S, D = 2, 2048, 1024
H, DK, DV = 16, 64, 64
HL = 4          # heads per core
NPAIR = 2       # head pairs per core (2 heads packed per 128 partitions)
ST = S // 128   # 16 s-tiles
DT = D // 128   # 8 d-tiles
SC = 512        # attention s-chunk
NSC = S // SC   # 4
N_CORES = 8


class _TileContextSplitDrain(tile.TileContext):
    """Walrus in this container rejects ANY instruction carrying >1 sem wait
    ("Too many sync wait commands"). Post-lowering, sweep every basic block
    and move surplus waits onto injected EventSemaphore carrier instructions
    placed immediately before the over-subscribed instruction (same engine,
    same program point - semantics unchanged). Also emit the final drain as
    one drain per logical proc so each carries a single wait."""

    _MAXW = 1

    def _split_excess_waits(self):
        nc = self.nc
        for fn in nc.m.functions:
            for bb in fn.blocks:
                insts = bb.instructions
                new_list = []
                changed = False
                for ins in insts:
                    si = ins.sync_info
                    waits = list(si.on_wait) if si is not None and si.on_wait else []
                    if len(waits) > self._MAXW:
                        changed = True
                        extra, keep = waits[:-self._MAXW], waits[-self._MAXW:]
                        for k in range(0, len(extra), self._MAXW):
                            chunk = extra[k:k + self._MAXW]
                            ev = mybir.InstEventSemaphore(
                                name=f"wsplit_{nc.next_id()}", ins=[], outs=[]
                            )
                            ev.engine = ins.engine
                            ev.sync_info = bass_rust.SyncInfo(
                                on_wait=chunk, on_update=[]
                            )
                            nc.register_instruction(ev, overwrite=True)
                            new_list.append(ev)
                        ins.sync_info = bass_rust.SyncInfo(
                            on_wait=keep,
                            on_update=list(si.on_update) if si.on_update else [],
                        )
                    new_list.append(ins)
                if changed:
                    insts[:] = new_list

    def _drain_and_barrier(self, tick_clock, wait_clock):
        self._split_excess_waits()
        ticks = list(tick_clock.global_clock)
        for p, t in enumerate(ticks):
            if t <= 0:
                continue
            v = bass_rust.VectorClock()
            v.require_at_least(p, t)
            d = self.nc.sync.drain()
            wait_clock.add_sem_waits(d.ins, ScopedClock({None: v}))
        self.nc.all_engine_barrier()
        popped = self.nc._tile_sem_poison_stack.pop()
        assert popped is self._sem_poison
        self.nc.clear_and_free_semaphores(list(self.sems.allocated().values()))
        self.nc.all_engine_barrier()


def build_nc() -> bass.Bass:
    nc = bass.Bass()

    xT_d = nc.dram_tensor("xT", [D, S], BF16, kind="ExternalInput")
    wqk_d = nc.dram_tensor("wqk", [D, 4 * 128], BF16, kind="ExternalInput")
    wv_d = nc.dram_tensor("wv", [D, HL * DV], BF16, kind="ExternalInput")
    wo_d = nc.dram_tensor("wo", [HL * DV, D], BF16, kind="ExternalInput")
    bqk_d = nc.dram_tensor("bqk", [128, 4], F32, kind="ExternalInput")
    bv_d = nc.dram_tensor("bv", [1, HL * DV], F32, kind="ExternalInput")
    bo4_d = nc.dram_tensor("bo4", [1, D], F32, kind="ExternalInput")
    out_d = nc.dram_tensor("out", [S, D], F32, kind="ExternalOutput")

    with _TileContextSplitDrain(nc) as tc:
        with (
            tc.tile_pool(name="const", bufs=1) as constp,
            tc.tile_pool(name="persist", bufs=1) as pers,
        ):
            wqk_sb = constp.tile([128, DT, 4 * 128], BF16, tag="wqk")
            wv_sb = constp.tile([128, DT, HL * DV], BF16, tag="wv")
            for d in range(DT):
                nc.sync.dma_start(out=wqk_sb[:, d, :], in_=wqk_d[d * 128:(d + 1) * 128, :])
                nc.sync.dma_start(out=wv_sb[:, d, :], in_=wv_d[d * 128:(d + 1) * 128, :])
            wo_sb = constp.tile([128, NPAIR, D], BF16, tag="wo")
            for p in range(NPAIR):
                nc.sync.dma_start(out=wo_sb[:, p, :], in_=wo_d[p * 128:(p + 1) * 128, :])
            bqk_sb = constp.tile([128, 4], F32, tag="bqk")
            nc.sync.dma_start(out=bqk_sb, in_=bqk_d[:, :])
            bv_rep = constp.tile([128, HL * DV], F32, tag="bv_rep")
            nc.sync.dma_start(out=bv_rep, in_=bv_d[0:1, :].to_broadcast((128, HL * DV)))
            bo_rep = constp.tile([128, D], F32, tag="bo_rep")
            nc.sync.dma_start(out=bo_rep, in_=bo4_d[0:1, :].to_broadcast((128, D)))

            # Persistent activation-side tensors
            xT_sb = pers.tile([128, DT, S], BF16, tag="xT")
            QT = pers.tile([128, NPAIR, S], BF16, tag="QT")
            KT = pers.tile([128, NPAIR, S], BF16, tag="KT")
            ctxT = pers.tile([128, NPAIR, S], BF16, tag="ctxT")
            # [t, j, h, 0:64] = V + bv; [t, j, h, 64:128] = 1.0
            V_aug = pers.tile([128, ST, HL, 128], BF16, tag="V_aug")
            nc.vector.memset(V_aug[:, :, :, 64:128], 1.0)

            # ---------------- Phase 1: QKV projections ----------------------
            with (
                tc.tile_pool(name="qkps", bufs=5, space="PSUM") as qkp,
                tc.tile_pool(name="vps", bufs=3, space="PSUM") as vp,
            ):
                for sc in range(4):
                    for d in range(DT):
                        nc.sync.dma_start(
                            out=xT_sb[:, d, sc * 512:(sc + 1) * 512],
                            in_=xT_d[d * 128:(d + 1) * 128, sc * 512:(sc + 1) * 512],
                        )
                    for proj, dest in ((0, QT), (1, KT)):
                        for pair in range(NPAIR):
                            col = (2 * proj + pair) * 128
                            ps = qkp.tile([128, 512], F32, tag="qkps")
                            for d in range(DT):
                                nc.tensor.matmul(
                                    ps[:, :],
                                    wqk_sb[:, d, col:col + 128],
                                    xT_sb[:, d, sc * 512:(sc + 1) * 512],
                                    start=(d == 0),
                                    stop=(d == DT - 1),
                                )
                            nc.vector.tensor_scalar_add(
                                dest[:, pair, sc * 512:(sc + 1) * 512],
                                ps[:, :],
                                bqk_sb[:, 2 * proj + pair:2 * proj + pair + 1],
                            )
                    for tt in range(4):
                        j = 4 * sc + tt
                        vps = vp.tile([128, HL * DV], F32, tag="vps")
                        for d in range(DT):
                            nc.tensor.matmul(
                                vps[:, :],
                                xT_sb[:, d, j * 128:(j + 1) * 128],
                                wv_sb[:, d, :],
                                start=(d == 0),
                                stop=(d == DT - 1),
                            )
                        nc.vector.tensor_add(
                            V_aug[:, j, :, 0:64],
                            vps.rearrange("p (h v) -> p h v", h=HL),
                            bv_rep.rearrange("p (h v) -> p h v", h=HL),
                        )

            # ---------------- Phase 2: attention + outproj, chunk-outer -----
            with (
                tc.tile_pool(name="sps", bufs=2, space="PSUM") as spp,
                tc.tile_pool(name="cpp", bufs=2, space="PSUM") as cpp,
                tc.tile_pool(name="opp", bufs=2, space="PSUM") as opp,
                tc.tile_pool(name="ptp", bufs=3) as ptp,
                tc.tile_pool(name="recp", bufs=2) as recp,
                tc.tile_pool(name="otp", bufs=3) as otp,
            ):
                for c in range(NSC):
                    for pair in range(NPAIR):
                        for e in range(2):
                            h = 2 * pair + e
                            base = 64 * e
                            cp = cpp.tile([128, SC], F32, name="cp", tag="cp")
                            sps = {}

                            def emit_scores(g, _pair=pair, _base=base, _c=c, _sps=None):
                                sp = spp.tile([128, 2, SC], F32, name="sp", tag="sp")
                                for k in range(2):
                                    j = 2 * g + k
                                    nc.tensor.matmul(
                                        sp[:, k, :],
                                        KT[_base:_base + 64, _pair, j * 128:(j + 1) * 128],
                                        QT[_base:_base + 64, _pair, _c * SC:(_c + 1) * SC],
                                        start=True,
                                        stop=True,
                                    )
                                sps[g] = sp

                            emit_scores(0)
                            emit_scores(1)
                            for g in range(8):
                                if g + 2 < 8:
                                    emit_scores(g + 2)
                                sp = sps.pop(g)
                                pt = ptp.tile([128, 2, SC], BF16, tag="pt")
                                nc.scalar.activation(
                                    out=pt[:, :, :], in_=sp[:, :, :], func=AF.Exp
                                )
                                for k in range(2):
                                    j = 2 * g + k
                                    nc.tensor.matmul(
                                        cp[:, :],
                                        V_aug[:, j, h, :],
                                        pt[:, k, :],
                                        start=(g == 0 and k == 0),
                                        stop=(g == 7 and k == 1),
                                    )
                            rec = recp.tile([64, SC], F32, tag="rec")
                            nc.vector.reciprocal(rec, cp[64:128, :])
                            nc.vector.tensor_mul(
                                ctxT[base:base + 64, pair, c * SC:(c + 1) * SC],
                                cp[0:64, :],
                                rec,
                            )
                    # output projection for this chunk's 4 s-tiles
                    for st in range(4 * c, 4 * c + 4):
                        ot = otp.tile([128, D], F32, tag="ot")
                        for dc in range(2):
                            ops_t = opp.tile([128, 512], F32, name="ops", tag="ops")
                            for pair in range(NPAIR):
                                nc.tensor.matmul(
                                    ops_t[:, :],
                                    ctxT[:, pair, st * 128:(st + 1) * 128],
                                    wo_sb[:, pair, dc * 512:(dc + 1) * 512],
                                    start=(pair == 0),
                                    stop=(pair == NPAIR - 1),
                                )
                            nc.vector.tensor_add(
                                ot[:, dc * 512:(dc + 1) * 512],
                                ops_t[:, :],
                                bo_rep[:, dc * 512:(dc + 1) * 512],
                            )
                        nc.sync.dma_start(out=out_d[st * 128:(st + 1) * 128, :], in_=ot)

    return nc


_NC_CACHE = None


def get_nc() -> bass.Bass:
    global _NC_CACHE
    if _NC_CACHE is None:
        _NC_CACHE = build_nc()
    return _NC_CACHE


def prep_in_maps(hidden_state, Wq, bq, Wk, bk, Wv, bv, Wo, bo):
    hidden_state = np.asarray(hidden_state, np.float32)
    Wq, bq = np.asarray(Wq, np.float32), np.asarray(bq, np.float32)
    Wk, bk = np.asarray(Wk, np.float32), np.asarray(bk, np.float32)
    Wv, bv = np.asarray(Wv, np.float32), np.asarray(bv, np.float32)
    Wo, bo = np.asarray(Wo, np.float32), np.asarray(bo, np.float32)
    scale = np.float32(1.0 / np.sqrt(DK))

    in_maps = []
    for core in range(N_CORES):
        b, g = core // 4, core % 4
        hs = slice(HL * g, HL * (g + 1))
        xT = np.ascontiguousarray(hidden_state[b].T).astype(BF16NP)
        # wqk cols: q-pair0 | q-pair1 | k-pair0 | k-pair1, each [D, 128]
        wq_g = Wq[hs] * scale                      # [4, D, 64]
        wk_g = Wk[hs]
        cols = []
        for wmat in (wq_g, wk_g):
            for pair in range(NPAIR):
                cols.append(
                    wmat[2 * pair:2 * pair + 2].transpose(1, 0, 2).reshape(D, 128)
                )
        wqk = np.concatenate(cols, axis=1).astype(BF16NP)
        wv_g = Wv[hs].transpose(1, 0, 2).reshape(D, HL * DV).astype(BF16NP)
        bqk_cols = []
        for bvec in (bq[hs] * scale, bk[hs]):
            for pair in range(NPAIR):
                bqk_cols.append(bvec[2 * pair:2 * pair + 2].reshape(128))
        bqk = np.stack(bqk_cols, axis=1).astype(np.float32)
        in_maps.append({
            "xT": xT,
            "wqk": np.ascontiguousarray(wqk),
            "wv": np.ascontiguousarray(wv_g),
            "wo": np.ascontiguousarray(Wo[HL * DV * g: HL * DV * (g + 1)]).astype(BF16NP),
            "bqk": np.ascontiguousarray(bqk),
            "bv": np.ascontiguousarray(bv[hs].reshape(1, HL * DV)),
            "bo4": np.ascontiguousarray((bo / 4.0)[None, :]),
        })
    return in_maps


def gather(results):
    """Sum the 4 row-parallel partials per batch."""
    out = np.empty((B, S, D), np.float32)
    for b in range(B):
        acc = results[4 * b]["out"].astype(np.float32)
        for g in range(1, 4):
            acc = acc + results[4 * b + g]["out"]
        out[b] = acc
    return out


def kernel(**inputs) -> np.ndarray:
    nc = get_nc()
    in_maps = prep_in_maps(**inputs)
    res = run_bass_kernel_spmd(nc, in_maps, core_ids=list(range(N_CORES)))
    return gather(res.results)


# revision 15
# speedup vs baseline: 1.6486x; 1.2050x over previous
"""Multi-head attention Trainium2 kernel (8 NeuronCores, SPMD).

Problem: B=2, S=2048, D=1024, H=16 heads, DK=DV=64.
Sharding: batch (2) x head-groups (4 heads per core) = 8 cores.
Each core computes, for its batch b and its 4 heads:
    Q/K/V projections, attention, and the partial output projection
    out_partial = concat_heads(ctx) @ Wo[head rows].
Host gathers by summing the 4 partials per batch and adding bo.

v4 design (405us baseline -> 298us v2 -> this). All matmuls bf16/fp32-psum:
fp8 anywhere on the q/k or value paths exceeds the 2e-2 max-rel-err budget
(measured 1e-2 for fp8 q/k alone, 3.5e-2 for fp8 ctx/Wo), and fp8 DoubleRow
gave no per-instruction speedup on this hardware anyway (~450ns vs ~340ns
per 512-row matmul).

Keys to the speedup over v2:
  - The PE runs at full clock only when continuously fed (the projection
    phase hits 2.4GHz; the v2 attention loop sagged to ~1.4GHz because the
    per-group exp dependency starves it ~200ns per group and the clock
    drops). The output projection of chunk c-1 is therefore INTERLEAVED
    into chunk c's attention stream as always-ready filler work.
  - exp is split across engines: groups 0,2,4,6 use exact Act-engine Exp;
    groups 1,3,5,7 use a DVE fast-exp (int16(s*23.083+16249.64) bit
    pattern read as bf16 ~= exp(s/8), 1.7% mean err; washes out in the
    2048-way softmax averages - measured end-to-end 7e-3).
  - 1/sqrt(DK) folds into the exp scale (and the fast-exp multiplier).
  - Softmax denominator comes free from 64 ones-columns appended to V
    (PSUM rows 64..127 of the ctx accumulator); its reciprocal is
    exp(-ln(den)) on Act - ln/exp share one activation table (no reload),
    vs 3.4us per tile for DVE RECIPROCAL.
  - x arrives host-pre-transposed in bf16; V is computed directly in
    [t,(h,v)] layout (stationary = xT tile) - no PE transposes at all.
  - Q/K-projection PSUM evacuation alternates Act (Identity+bias) / DVE
    (tensor_scalar_add); outproj evacuation alternates Act/DVE copies;
    bo is added by the host during the gather.
"""
import sys

if "/opt/trn_rl_repo" not in sys.path:
    sys.path.insert(0, "/opt/trn_rl_repo")

import ml_dtypes
import numpy as np

import bass_rust
import concourse.bass as bass
import concourse.mybir as mybir
import concourse.tile as tile
from concourse.bass_utils import run_bass_kernel_spmd
from concourse.vector_clock import ScopedClock

F32 = mybir.dt.float32
BF16 = mybir.dt.bfloat16
I16 = mybir.dt.int16
AF = mybir.ActivationFunctionType
Alu = mybir.AluOpType
BF16NP = ml_dtypes.bfloat16

B, S, D = 2, 2048, 1024
H, DK, DV = 16, 64, 64
HL = 4          # heads per core
NPAIR = 2
ST = S // 128   # 16
DT = D // 128   # 8
SC = 512        # attention s-chunk
NSC = S // SC   # 4
N_CORES = 8

# int16(score*FEXP_A + FEXP_B) bit pattern read as bf16 ~= exp(score/8)
FEXP_A = 0.125 * 128.0 * 1.4426950408889634
FEXP_B = 16249.64
ACT_GROUPS = (0, 2, 4, 6)   # exact Act exp; remaining groups use DVE fastexp


class _TileContextSplitDrain(tile.TileContext):
    """Walrus in this container rejects ANY instruction carrying >1 sem wait
    ("Too many sync wait commands"). Post-lowering, sweep every basic block
    and move surplus waits onto injected EventSemaphore carrier instructions
    placed immediately before the over-subscribed instruction (same engine,
    same program point - semantics unchanged)."""

    _MAXW = 1

    def _split_excess_waits(self):
        nc = self.nc
        for fn in nc.m.functions:
            for bb in fn.blocks:
                insts = bb.instructions
                new_list = []
                changed = False
                for ins in insts:
                    si = ins.sync_info
                    waits = list(si.on_wait) if si is not None and si.on_wait else []
                    if len(waits) > self._MAXW:
                        changed = True
                        extra, keep = waits[:-self._MAXW], waits[-self._MAXW:]
                        for k in range(0, len(extra), self._MAXW):
                            chunk = extra[k:k + self._MAXW]
                            ev = mybir.InstEventSemaphore(
                                name=f"wsplit_{nc.next_id()}", ins=[], outs=[]
                            )
                            ev.engine = ins.engine
                            ev.sync_info = bass_rust.SyncInfo(
                                on_wait=chunk, on_update=[]
                            )
                            nc.register_instruction(ev, overwrite=True)
                            new_list.append(ev)
                        ins.sync_info = bass_rust.SyncInfo(
                            on_wait=keep,
                            on_update=list(si.on_update) if si.on_update else [],
                        )
                    new_list.append(ins)
                if changed:
                    insts[:] = new_list

    def _drain_and_barrier(self, tick_clock, wait_clock):
        self._split_excess_waits()
        ticks = list(tick_clock.global_clock)
        for p, t in enumerate(ticks):
            if t <= 0:
                continue
            v = bass_rust.VectorClock()
            v.require_at_least(p, t)
            d = self.nc.sync.drain()
            wait_clock.add_sem_waits(d.ins, ScopedClock({None: v}))
        self.nc.all_engine_barrier()
        popped = self.nc._tile_sem_poison_stack.pop()
        assert popped is self._sem_poison
        self.nc.clear_and_free_semaphores(list(self.sems.allocated().values()))
        self.nc.all_engine_barrier()


def build_nc() -> bass.Bass:
    nc = bass.Bass()

    xT_d = nc.dram_tensor("xT", [D, S], BF16, kind="ExternalInput")
    wqk_d = nc.dram_tensor("wqk", [D, 4 * 128], BF16, kind="ExternalInput")
    wv_d = nc.dram_tensor("wv", [D, HL * DV], BF16, kind="ExternalInput")
    wo_d = nc.dram_tensor("wo", [HL * DV, D], BF16, kind="ExternalInput")
    bqk_d = nc.dram_tensor("bqk", [128, 4], F32, kind="ExternalInput")
    bv_d = nc.dram_tensor("bv", [1, HL * DV], F32, kind="ExternalInput")
    out_d = nc.dram_tensor("out", [S, D], F32, kind="ExternalOutput")

    with _TileContextSplitDrain(nc) as tc:
        with (
            tc.tile_pool(name="const", bufs=1) as constp,
            tc.tile_pool(name="persist", bufs=1) as pers,
        ):
            wqk_sb = constp.tile([128, DT, 4 * 128], BF16, tag="wqk")
            wv_sb = constp.tile([128, DT, HL * DV], BF16, tag="wv")
            for d in range(DT):
                nc.sync.dma_start(out=wqk_sb[:, d, :], in_=wqk_d[d * 128:(d + 1) * 128, :])
                nc.sync.dma_start(out=wv_sb[:, d, :], in_=wv_d[d * 128:(d + 1) * 128, :])
            wo_sb = constp.tile([128, NPAIR, D], BF16, tag="wo")
            for p in range(NPAIR):
                nc.sync.dma_start(out=wo_sb[:, p, :], in_=wo_d[p * 128:(p + 1) * 128, :])
            bqk_sb = constp.tile([128, 4], F32, tag="bqk")
            nc.sync.dma_start(out=bqk_sb, in_=bqk_d[:, :])
            bv_rep = constp.tile([128, HL * DV], F32, tag="bv_rep")
            nc.sync.dma_start(out=bv_rep, in_=bv_d[0:1, :].to_broadcast((128, HL * DV)))

            xT_sb = pers.tile([128, DT, S], BF16, tag="xT")
            QT = pers.tile([128, NPAIR, S], BF16, tag="QT")
            KT = pers.tile([128, NPAIR, S], BF16, tag="KT")
            ctxT = pers.tile([128, NPAIR, S], BF16, tag="ctxT")
            V_aug = pers.tile([128, ST, HL, 128], BF16, tag="V_aug")
            nc.vector.memset(V_aug[:, :, :, 64:128], 1.0)

            # ---------------- Phase 1: QKV projections ----------------------
            with (
                tc.tile_pool(name="qkps", bufs=5, space="PSUM") as qkp,
                tc.tile_pool(name="vps", bufs=3, space="PSUM") as vp,
            ):
                for sc in range(4):
                    for d in range(DT):
                        nc.sync.dma_start(
                            out=xT_sb[:, d, sc * 512:(sc + 1) * 512],
                            in_=xT_d[d * 128:(d + 1) * 128, sc * 512:(sc + 1) * 512],
                        )
                    for proj, dest in ((0, QT), (1, KT)):
                        for pair in range(NPAIR):
                            col = (2 * proj + pair) * 128
                            ps = qkp.tile([128, 512], F32, tag="qkps")
                            for d in range(DT):
                                nc.tensor.matmul(
                                    ps[:, :],
                                    wqk_sb[:, d, col:col + 128],
                                    xT_sb[:, d, sc * 512:(sc + 1) * 512],
                                    start=(d == 0),
                                    stop=(d == DT - 1),
                                )
                            dst = dest[:, pair, sc * 512:(sc + 1) * 512]
                            bias = bqk_sb[:, 2 * proj + pair:2 * proj + pair + 1]
                            if pair == 0:
                                nc.scalar.activation(
                                    out=dst, in_=ps[:, :], func=AF.Identity,
                                    bias=bias, scale=1.0,
                                )
                            else:
                                nc.vector.tensor_scalar_add(dst, ps[:, :], bias)
                    for tt in range(4):
                        j = 4 * sc + tt
                        vps = vp.tile([128, HL * DV], F32, tag="vps")
                        for d in range(DT):
                            nc.tensor.matmul(
                                vps[:, :],
                                xT_sb[:, d, j * 128:(j + 1) * 128],
                                wv_sb[:, d, :],
                                start=(d == 0),
                                stop=(d == DT - 1),
                            )
                        nc.vector.tensor_add(
                            V_aug[:, j, :, 0:64],
                            vps.rearrange("p (h v) -> p h v", h=HL),
                            bv_rep.rearrange("p (h v) -> p h v", h=HL),
                        )

            # ---------------- Phase 2: attention with interleaved outproj ---
            with (
                tc.tile_pool(name="sps", bufs=2, space="PSUM") as spp,
                tc.tile_pool(name="cpp", bufs=2, space="PSUM") as cpp,
                tc.tile_pool(name="opp", bufs=2, space="PSUM") as opp,
                tc.tile_pool(name="ptp", bufs=3) as ptp,
                tc.tile_pool(name="recl", bufs=2) as recl,
                tc.tile_pool(name="recp", bufs=2) as recp,
                tc.tile_pool(name="otp", bufs=3) as otp,
            ):
                pending = []   # outproj steps of the previous chunk
                ot_ref = [None]

                def outproj_step(st, dc):
                    if dc == 0:
                        ot_ref[0] = otp.tile([128, D], F32, name="ot", tag="ot")
                    ot = ot_ref[0]
                    ops_t = opp.tile([128, 512], F32, name="ops", tag="ops")
                    for pair in range(NPAIR):
                        nc.tensor.matmul(
                            ops_t[:, :],
                            ctxT[:, pair, st * 128:(st + 1) * 128],
                            wo_sb[:, pair, dc * 512:(dc + 1) * 512],
                            start=(pair == 0),
                            stop=(pair == NPAIR - 1),
                        )
                    if dc == 0:
                        nc.scalar.copy(ot[:, 0:512], ops_t[:, :])
                    else:
                        nc.vector.tensor_copy(ot[:, 512:1024], ops_t[:, :])
                        nc.sync.dma_start(
                            out=out_d[st * 128:(st + 1) * 128, :], in_=ot
                        )

                for c in range(NSC):
                    for pair in range(NPAIR):
                        for e in range(2):
                            h = 2 * pair + e
                            base = 64 * e
                            cp = cpp.tile([128, SC], F32, name="cp", tag="cp")
                            sps = {}

                            def emit_scores(g, _pair=pair, _base=base, _c=c):
                                sp = spp.tile([128, 2, SC], F32, name="sp", tag="sp")
                                for k in range(2):
                                    j = 2 * g + k
                                    nc.tensor.matmul(
                                        sp[:, k, :],
                                        KT[_base:_base + 64, _pair, j * 128:(j + 1) * 128],
                                        QT[_base:_base + 64, _pair, _c * SC:(_c + 1) * SC],
                                        start=True,
                                        stop=True,
                                    )
                                sps[g] = sp

                            emit_scores(0)
                            emit_scores(1)
                            for g in range(8):
                                if g + 2 < 8:
                                    emit_scores(g + 2)
                                sp = sps.pop(g)
                                pt = ptp.tile([128, 2, SC], BF16, tag="pt")
                                if g in ACT_GROUPS:
                                    nc.scalar.activation(
                                        out=pt[:, :, :], in_=sp[:, :, :],
                                        func=AF.Exp, scale=0.125,
                                    )
                                else:
                                    nc.vector.tensor_scalar(
                                        pt.bitcast(I16), sp[:, :, :],
                                        FEXP_A, FEXP_B, Alu.mult, Alu.add,
                                    )
                                for k in range(2):
                                    j = 2 * g + k
                                    nc.tensor.matmul(
                                        cp[:, :],
                                        V_aug[:, j, h, :],
                                        pt[:, k, :],
                                        start=(g == 0 and k == 0),
                                        stop=(g == 7 and k == 1),
                                    )
                                if g % 2 == 1 and pending:
                                    pending.pop(0)()
                            # 1/den = exp(-ln(den)); den sits on psum rows
                            # 64..127 via the ones-columns of V_aug
                            rl = recl.tile([64, SC], F32, tag="rl")
                            nc.scalar.activation(out=rl, in_=cp[64:128, :], func=AF.Ln)
                            rec = recp.tile([64, SC], F32, tag="rec")
                            nc.scalar.activation(out=rec, in_=rl, func=AF.Exp, scale=-1.0)
                            nc.vector.tensor_mul(
                                ctxT[base:base + 64, pair, c * SC:(c + 1) * SC],
                                cp[0:64, :],
                                rec,
                            )
                    for st in range(4 * c, 4 * c + 4):
                        for dc in range(2):
                            pending.append(
                                lambda _st=st, _dc=dc: outproj_step(_st, _dc)
                            )
                while pending:
                    pending.pop(0)()

    return nc


_NC_CACHE = None


def get_nc() -> bass.Bass:
    global _NC_CACHE
    if _NC_CACHE is None:
        _NC_CACHE = build_nc()
    return _NC_CACHE


def prep_in_maps(hidden_state, Wq, bq, Wk, bk, Wv, bv, Wo, bo):
    hidden_state = np.asarray(hidden_state, np.float32)
    Wq, bq = np.asarray(Wq, np.float32), np.asarray(bq, np.float32)
    Wk, bk = np.asarray(Wk, np.float32), np.asarray(bk, np.float32)
    Wv, bv = np.asarray(Wv, np.float32), np.asarray(bv, np.float32)
    Wo, bo = np.asarray(Wo, np.float32), np.asarray(bo, np.float32)

    in_maps = []
    for core in range(N_CORES):
        b, g = core // 4, core % 4
        hs = slice(HL * g, HL * (g + 1))
        xT = np.ascontiguousarray(hidden_state[b].T).astype(BF16NP)
        cols = []
        for wmat in (Wq[hs], Wk[hs]):
            for pair in range(NPAIR):
                cols.append(
                    wmat[2 * pair:2 * pair + 2].transpose(1, 0, 2).reshape(D, 128)
                )
        wqk = np.concatenate(cols, axis=1).astype(BF16NP)
        wv_g = Wv[hs].transpose(1, 0, 2).reshape(D, HL * DV).astype(BF16NP)
        bqk_cols = []
        for bvec in (bq[hs], bk[hs]):
            for pair in range(NPAIR):
                bqk_cols.append(bvec[2 * pair:2 * pair + 2].reshape(128))
        bqk = np.stack(bqk_cols, axis=1).astype(np.float32)
        in_maps.append({
            "xT": xT,
            "wqk": np.ascontiguousarray(wqk),
            "wv": np.ascontiguousarray(wv_g),
            "wo": np.ascontiguousarray(Wo[HL * DV * g: HL * DV * (g + 1)]).astype(BF16NP),
            "bqk": np.ascontiguousarray(bqk),
            "bv": np.ascontiguousarray(bv[hs].reshape(1, HL * DV)),
        })
    return in_maps


_BO = None


def gather(results):
    """Sum the 4 row-parallel partials per batch, then add bo."""
    out = np.empty((B, S, D), np.float32)
    for b in range(B):
        acc = results[4 * b]["out"].astype(np.float32)
        for g in range(1, 4):
            acc = acc + results[4 * b + g]["out"]
        out[b] = acc + _BO[None, :]
    return out


def kernel(**inputs) -> np.ndarray:
    global _BO
    _BO = np.asarray(inputs["bo"], np.float32)
    nc = get_nc()
    in_maps = prep_in_maps(**inputs)
    res = run_bass_kernel_spmd(nc, in_maps, core_ids=list(range(N_CORES)))
    return gather(res.results)


# revision 16
# speedup vs baseline: 1.8006x; 1.0922x over previous
"""Multi-head attention Trainium2 kernel (8 NeuronCores, SPMD).

Problem: B=2, S=2048, D=1024, H=16 heads, DK=DV=64.
Sharding: batch (2) x head-groups (4 heads per core) = 8 cores.
Each core computes, for its batch b and its 4 heads:
    Q/K/V projections, attention, and the partial output projection
    out_partial = concat_heads(ctx) @ Wo[head rows].
Host gathers by summing the 4 partials per batch and adding bo.

v4 design (405us baseline -> 298us v2 -> this). All matmuls bf16/fp32-psum:
fp8 anywhere on the q/k or value paths exceeds the 2e-2 max-rel-err budget
(measured 1e-2 for fp8 q/k alone, 3.5e-2 for fp8 ctx/Wo), and fp8 DoubleRow
gave no per-instruction speedup on this hardware anyway (~450ns vs ~340ns
per 512-row matmul).

Keys to the speedup over v2:
  - The PE runs at full clock only when continuously fed (the projection
    phase hits 2.4GHz; the v2 attention loop sagged to ~1.4GHz because the
    per-group exp dependency starves it ~200ns per group and the clock
    drops). The output projection of chunk c-1 is therefore INTERLEAVED
    into chunk c's attention stream as always-ready filler work.
  - exp is split across engines: groups 0,2,4,6 use exact Act-engine Exp;
    groups 1,3,5,7 use a DVE fast-exp (int16(s*23.083+16249.64) bit
    pattern read as bf16 ~= exp(s/8), 1.7% mean err; washes out in the
    2048-way softmax averages - measured end-to-end 7e-3).
  - 1/sqrt(DK) folds into the exp scale (and the fast-exp multiplier).
  - Softmax denominator comes free from 64 ones-columns appended to V
    (PSUM rows 64..127 of the ctx accumulator); its reciprocal is
    exp(-ln(den)) on Act - ln/exp share one activation table (no reload),
    vs 3.4us per tile for DVE RECIPROCAL.
  - x arrives host-pre-transposed in bf16; V is computed directly in
    [t,(h,v)] layout (stationary = xT tile) - no PE transposes at all.
  - Q/K-projection PSUM evacuation alternates Act (Identity+bias) / DVE
    (tensor_scalar_add); outproj evacuation alternates Act/DVE copies;
    bo is added by the host during the gather.
"""
import sys

if "/opt/trn_rl_repo" not in sys.path:
    sys.path.insert(0, "/opt/trn_rl_repo")

import ml_dtypes
import numpy as np

import bass_rust
import concourse.bass as bass
import concourse.mybir as mybir
import concourse.tile as tile
from concourse.bass_utils import run_bass_kernel_spmd
from concourse.vector_clock import ScopedClock

F32 = mybir.dt.float32
BF16 = mybir.dt.bfloat16
I16 = mybir.dt.int16
AF = mybir.ActivationFunctionType
Alu = mybir.AluOpType
BF16NP = ml_dtypes.bfloat16

B, S, D = 2, 2048, 1024
H, DK, DV = 16, 64, 64
HL = 4          # heads per core
NPAIR = 2
ST = S // 128   # 16
DT = D // 128   # 8
SC = 512        # attention s-chunk
NSC = S // SC   # 4
N_CORES = 8

# int16(score*FEXP_A + FEXP_B) bit pattern read as bf16 ~= exp(score/8)
FEXP_A = 0.125 * 128.0 * 1.4426950408889634
FEXP_B = 16249.64
ACT_GROUPS = (0, 2, 4, 6)   # exact Act exp; remaining groups use DVE fastexp


class _TileContextSplitDrain(tile.TileContext):
    """Walrus in this container rejects ANY instruction carrying >1 sem wait
    ("Too many sync wait commands"). Post-lowering, sweep every basic block
    and move surplus waits onto injected EventSemaphore carrier instructions
    placed immediately before the over-subscribed instruction (same engine,
    same program point - semantics unchanged)."""

    _MAXW = 1

    def _split_excess_waits(self):
        nc = self.nc
        for fn in nc.m.functions:
            for bb in fn.blocks:
                insts = bb.instructions
                new_list = []
                changed = False
                for ins in insts:
                    si = ins.sync_info
                    waits = list(si.on_wait) if si is not None and si.on_wait else []
                    if len(waits) > self._MAXW:
                        changed = True
                        extra, keep = waits[:-self._MAXW], waits[-self._MAXW:]
                        for k in range(0, len(extra), self._MAXW):
                            chunk = extra[k:k + self._MAXW]
                            ev = mybir.InstEventSemaphore(
                                name=f"wsplit_{nc.next_id()}", ins=[], outs=[]
                            )
                            ev.engine = ins.engine
                            ev.sync_info = bass_rust.SyncInfo(
                                on_wait=chunk, on_update=[]
                            )
                            nc.register_instruction(ev, overwrite=True)
                            new_list.append(ev)
                        ins.sync_info = bass_rust.SyncInfo(
                            on_wait=keep,
                            on_update=list(si.on_update) if si.on_update else [],
                        )
                    new_list.append(ins)
                if changed:
                    insts[:] = new_list

    def _drain_and_barrier(self, tick_clock, wait_clock):
        self._split_excess_waits()
        ticks = list(tick_clock.global_clock)
        for p, t in enumerate(ticks):
            if t <= 0:
                continue
            v = bass_rust.VectorClock()
            v.require_at_least(p, t)
            d = self.nc.sync.drain()
            wait_clock.add_sem_waits(d.ins, ScopedClock({None: v}))
        self.nc.all_engine_barrier()
        popped = self.nc._tile_sem_poison_stack.pop()
        assert popped is self._sem_poison
        self.nc.clear_and_free_semaphores(list(self.sems.allocated().values()))
        self.nc.all_engine_barrier()


def build_nc() -> bass.Bass:
    nc = bass.Bass()

    # all inputs partition-major: row p holds everything partition p needs,
    # contiguously, so each dma_start is 128 large descriptors
    xT_d = nc.dram_tensor("xT", [128, NSC * DT * 512], BF16, kind="ExternalInput")
    wqk_d = nc.dram_tensor("wqk", [128, DT * 512], BF16, kind="ExternalInput")
    wv_d = nc.dram_tensor("wv", [128, DT * HL * DV], BF16, kind="ExternalInput")
    wo_d = nc.dram_tensor("wo", [128, NPAIR * D], BF16, kind="ExternalInput")
    bqk_d = nc.dram_tensor("bqk", [128, 4], F32, kind="ExternalInput")
    bv_d = nc.dram_tensor("bv", [1, HL * DV], F32, kind="ExternalInput")
    out_d = nc.dram_tensor("out", [S, D], F32, kind="ExternalOutput")

    with _TileContextSplitDrain(nc) as tc:
        with (
            tc.tile_pool(name="const", bufs=1) as constp,
            tc.tile_pool(name="persist", bufs=1) as pers,
        ):
            wqk_sb = constp.tile([128, DT, 4 * 128], BF16, tag="wqk")
            wv_sb = constp.tile([128, DT, HL * DV], BF16, tag="wv")
            wo_sb = constp.tile([128, NPAIR, D], BF16, tag="wo")
            bqk_sb = constp.tile([128, 4], F32, tag="bqk")
            bv_rep = constp.tile([128, HL * DV], F32, tag="bv_rep")
            xT_sb = pers.tile([128, DT, S], BF16, tag="xT")
            QT = pers.tile([128, NPAIR, S], BF16, tag="QT")
            KT = pers.tile([128, NPAIR, S], BF16, tag="KT")
            ctxT = pers.tile([128, NPAIR, S], BF16, tag="ctxT")
            V_aug = pers.tile([128, ST, HL, 128], BF16, tag="V_aug")

            def dma_xT(sc):
                nc.sync.dma_start(
                    out=xT_sb[:, :, sc * 512:(sc + 1) * 512],
                    in_=xT_d.rearrange("p (c d s) -> p c d s", c=NSC, d=DT)[:, sc, :, :],
                )

            dma_xT(0)
            nc.sync.dma_start(out=wqk_sb, in_=wqk_d[:, :])
            dma_xT(1)
            nc.sync.dma_start(out=wv_sb, in_=wv_d[:, :])
            nc.sync.dma_start(out=bqk_sb, in_=bqk_d[:, :])
            nc.sync.dma_start(out=bv_rep, in_=bv_d[0:1, :].to_broadcast((128, HL * DV)))
            nc.vector.memset(V_aug[:, :, :, 64:128], 1.0)

            # ---------------- Phase 1: QKV projections ----------------------
            with (
                tc.tile_pool(name="qkps", bufs=5, space="PSUM") as qkp,
                tc.tile_pool(name="vps", bufs=3, space="PSUM") as vp,
            ):
                for sc in range(4):
                    if sc + 2 < 4:
                        dma_xT(sc + 2)
                    if sc == 1:
                        nc.sync.dma_start(out=wo_sb, in_=wo_d[:, :])
                    for proj, dest in ((0, QT), (1, KT)):
                        for pair in range(NPAIR):
                            col = (2 * proj + pair) * 128
                            ps = qkp.tile([128, 512], F32, tag="qkps")
                            for d in range(DT):
                                nc.tensor.matmul(
                                    ps[:, :],
                                    wqk_sb[:, d, col:col + 128],
                                    xT_sb[:, d, sc * 512:(sc + 1) * 512],
                                    start=(d == 0),
                                    stop=(d == DT - 1),
                                )
                            dst = dest[:, pair, sc * 512:(sc + 1) * 512]
                            bias = bqk_sb[:, 2 * proj + pair:2 * proj + pair + 1]
                            if pair == 0:
                                nc.scalar.activation(
                                    out=dst, in_=ps[:, :], func=AF.Identity,
                                    bias=bias, scale=1.0,
                                )
                            else:
                                nc.vector.tensor_scalar_add(dst, ps[:, :], bias)
                    for tt in range(4):
                        j = 4 * sc + tt
                        vps = vp.tile([128, HL * DV], F32, tag="vps")
                        for d in range(DT):
                            nc.tensor.matmul(
                                vps[:, :],
                                xT_sb[:, d, j * 128:(j + 1) * 128],
                                wv_sb[:, d, :],
                                start=(d == 0),
                                stop=(d == DT - 1),
                            )
                        nc.vector.tensor_add(
                            V_aug[:, j, :, 0:64],
                            vps.rearrange("p (h v) -> p h v", h=HL),
                            bv_rep.rearrange("p (h v) -> p h v", h=HL),
                        )

            # ---------------- Phase 2: attention with interleaved outproj ---
            with (
                tc.tile_pool(name="sps", bufs=2, space="PSUM") as spp,
                tc.tile_pool(name="cpp", bufs=2, space="PSUM") as cpp,
                tc.tile_pool(name="opp", bufs=2, space="PSUM") as opp,
                tc.tile_pool(name="ptp", bufs=3) as ptp,
                tc.tile_pool(name="recl", bufs=2) as recl,
                tc.tile_pool(name="recp", bufs=2) as recp,
                tc.tile_pool(name="otp", bufs=3) as otp,
            ):
                pending = []   # outproj steps of the previous chunk
                ot_ref = [None]

                def outproj_step(st, dc):
                    if dc == 0:
                        ot_ref[0] = otp.tile([128, D], F32, name="ot", tag="ot")
                    ot = ot_ref[0]
                    ops_t = opp.tile([128, 512], F32, name="ops", tag="ops")
                    for pair in range(NPAIR):
                        nc.tensor.matmul(
                            ops_t[:, :],
                            ctxT[:, pair, st * 128:(st + 1) * 128],
                            wo_sb[:, pair, dc * 512:(dc + 1) * 512],
                            start=(pair == 0),
                            stop=(pair == NPAIR - 1),
                        )
                    if dc == 0:
                        nc.scalar.copy(ot[:, 0:512], ops_t[:, :])
                    else:
                        nc.vector.tensor_copy(ot[:, 512:1024], ops_t[:, :])
                        nc.sync.dma_start(
                            out=out_d[st * 128:(st + 1) * 128, :], in_=ot
                        )

                for c in range(NSC):
                    for pair in range(NPAIR):
                        for e in range(2):
                            h = 2 * pair + e
                            base = 64 * e
                            cp = cpp.tile([128, SC], F32, name="cp", tag="cp")
                            sps = {}

                            def emit_scores(g, _pair=pair, _base=base, _c=c):
                                sp = spp.tile([128, 2, SC], F32, name="sp", tag="sp")
                                for k in range(2):
                                    j = 2 * g + k
                                    nc.tensor.matmul(
                                        sp[:, k, :],
                                        KT[_base:_base + 64, _pair, j * 128:(j + 1) * 128],
                                        QT[_base:_base + 64, _pair, _c * SC:(_c + 1) * SC],
                                        start=True,
                                        stop=True,
                                    )
                                sps[g] = sp

                            emit_scores(0)
                            emit_scores(1)
                            for g in range(8):
                                if g + 2 < 8:
                                    emit_scores(g + 2)
                                sp = sps.pop(g)
                                pt = ptp.tile([128, 2, SC], BF16, tag="pt")
                                if g in ACT_GROUPS:
                                    nc.scalar.activation(
                                        out=pt[:, :, :], in_=sp[:, :, :],
                                        func=AF.Exp, scale=0.125,
                                    )
                                else:
                                    nc.vector.tensor_scalar(
                                        pt.bitcast(I16), sp[:, :, :],
                                        FEXP_A, FEXP_B, Alu.mult, Alu.add,
                                    )
                                for k in range(2):
                                    j = 2 * g + k
                                    nc.tensor.matmul(
                                        cp[:, :],
                                        V_aug[:, j, h, :],
                                        pt[:, k, :],
                                        start=(g == 0 and k == 0),
                                        stop=(g == 7 and k == 1),
                                    )
                                if g % 2 == 1 and pending:
                                    pending.pop(0)()
                            # 1/den = exp(-ln(den)); den sits on psum rows
                            # 64..127 via the ones-columns of V_aug
                            rl = recl.tile([64, SC], F32, tag="rl")
                            nc.scalar.activation(out=rl, in_=cp[64:128, :], func=AF.Ln)
                            rec = recp.tile([64, SC], F32, tag="rec")
                            nc.scalar.activation(out=rec, in_=rl, func=AF.Exp, scale=-1.0)
                            nc.vector.tensor_mul(
                                ctxT[base:base + 64, pair, c * SC:(c + 1) * SC],
                                cp[0:64, :],
                                rec,
                            )
                    for st in range(4 * c, 4 * c + 4):
                        for dc in range(2):
                            pending.append(
                                lambda _st=st, _dc=dc: outproj_step(_st, _dc)
                            )
                while pending:
                    pending.pop(0)()

    return nc


_NC_CACHE = None


def get_nc() -> bass.Bass:
    global _NC_CACHE
    if _NC_CACHE is None:
        _NC_CACHE = build_nc()
    return _NC_CACHE


def prep_in_maps(hidden_state, Wq, bq, Wk, bk, Wv, bv, Wo, bo):
    hidden_state = np.asarray(hidden_state, np.float32)
    Wq, bq = np.asarray(Wq, np.float32), np.asarray(bq, np.float32)
    Wk, bk = np.asarray(Wk, np.float32), np.asarray(bk, np.float32)
    Wv, bv = np.asarray(Wv, np.float32), np.asarray(bv, np.float32)
    Wo, bo = np.asarray(Wo, np.float32), np.asarray(bo, np.float32)

    in_maps = []
    for core in range(N_CORES):
        b, g = core // 4, core % 4
        hs = slice(HL * g, HL * (g + 1))
        # [D, C] -> [128, DT, C]: row p holds d-tiles d*128+p contiguously
        def pmaj(a):
            return np.ascontiguousarray(
                a.reshape(DT, 128, -1).transpose(1, 0, 2)
            ).reshape(128, -1)

        xT = hidden_state[b].T.astype(BF16NP)
        # [128, NSC, DT, 512]
        xT = np.ascontiguousarray(
            xT.reshape(DT, 128, NSC, 512).transpose(1, 2, 0, 3)
        ).reshape(128, -1)
        cols = []
        for wmat in (Wq[hs], Wk[hs]):
            for pair in range(NPAIR):
                cols.append(
                    wmat[2 * pair:2 * pair + 2].transpose(1, 0, 2).reshape(D, 128)
                )
        wqk = pmaj(np.concatenate(cols, axis=1).astype(BF16NP))
        wv_g = pmaj(Wv[hs].transpose(1, 0, 2).reshape(D, HL * DV).astype(BF16NP))
        wo_g = np.ascontiguousarray(
            Wo[HL * DV * g: HL * DV * (g + 1)].astype(BF16NP)
            .reshape(NPAIR, 128, D).transpose(1, 0, 2)
        ).reshape(128, -1)
        bqk_cols = []
        for bvec in (bq[hs], bk[hs]):
            for pair in range(NPAIR):
                bqk_cols.append(bvec[2 * pair:2 * pair + 2].reshape(128))
        bqk = np.stack(bqk_cols, axis=1).astype(np.float32)
        in_maps.append({
            "xT": xT,
            "wqk": np.ascontiguousarray(wqk),
            "wv": np.ascontiguousarray(wv_g),
            "wo": wo_g,
            "bqk": np.ascontiguousarray(bqk),
            "bv": np.ascontiguousarray(bv[hs].reshape(1, HL * DV)),
        })
    return in_maps


_BO = None


def gather(results):
    """Sum the 4 row-parallel partials per batch, then add bo."""
    out = np.empty((B, S, D), np.float32)
    for b in range(B):
        acc = results[4 * b]["out"].astype(np.float32)
        for g in range(1, 4):
            acc = acc + results[4 * b + g]["out"]
        out[b] = acc + _BO[None, :]
    return out


def kernel(**inputs) -> np.ndarray:
    global _BO
    _BO = np.asarray(inputs["bo"], np.float32)
    nc = get_nc()
    in_maps = prep_in_maps(**inputs)
    res = run_bass_kernel_spmd(nc, in_maps, core_ids=list(range(N_CORES)))
    return gather(res.results)


# revision 17
# speedup vs baseline: 1.8110x; 1.0058x over previous
"""Multi-head attention Trainium2 kernel (8 NeuronCores, SPMD).

Problem: B=2, S=2048, D=1024, H=16 heads, DK=DV=64.
Sharding: batch (2) x head-groups (4 heads per core) = 8 cores.
Each core computes, for its batch b and its 4 heads:
    Q/K/V projections, attention, and the partial output projection
    out_partial = concat_heads(ctx) @ Wo[head rows].
Host gathers by summing the 4 partials per batch and adding bo.

v4 design (405us baseline -> 298us v2 -> this). All matmuls bf16/fp32-psum:
fp8 anywhere on the q/k or value paths exceeds the 2e-2 max-rel-err budget
(measured 1e-2 for fp8 q/k alone, 3.5e-2 for fp8 ctx/Wo), and fp8 DoubleRow
gave no per-instruction speedup on this hardware anyway (~450ns vs ~340ns
per 512-row matmul).

Keys to the speedup over v2:
  - The PE runs at full clock only when continuously fed (the projection
    phase hits 2.4GHz; the v2 attention loop sagged to ~1.4GHz because the
    per-group exp dependency starves it ~200ns per group and the clock
    drops). The output projection of chunk c-1 is therefore INTERLEAVED
    into chunk c's attention stream as always-ready filler work.
  - exp is split across engines: groups 0,2,4,6 use exact Act-engine Exp;
    groups 1,3,5,7 use a DVE fast-exp (int16(s*23.083+16249.64) bit
    pattern read as bf16 ~= exp(s/8), 1.7% mean err; washes out in the
    2048-way softmax averages - measured end-to-end 7e-3).
  - 1/sqrt(DK) folds into the exp scale (and the fast-exp multiplier).
  - Softmax denominator comes free from 64 ones-columns appended to V
    (PSUM rows 64..127 of the ctx accumulator); its reciprocal is
    exp(-ln(den)) on Act - ln/exp share one activation table (no reload),
    vs 3.4us per tile for DVE RECIPROCAL.
  - x arrives host-pre-transposed in bf16; V is computed directly in
    [t,(h,v)] layout (stationary = xT tile) - no PE transposes at all.
  - Q/K-projection PSUM evacuation alternates Act (Identity+bias) / DVE
    (tensor_scalar_add); outproj evacuation alternates Act/DVE copies;
    bo is added by the host during the gather.
"""
import sys

if "/opt/trn_rl_repo" not in sys.path:
    sys.path.insert(0, "/opt/trn_rl_repo")

import ml_dtypes
import numpy as np

import bass_rust
import concourse.bass as bass
import concourse.mybir as mybir
import concourse.tile as tile
from concourse.bass_utils import run_bass_kernel_spmd
from concourse.vector_clock import ScopedClock

F32 = mybir.dt.float32
BF16 = mybir.dt.bfloat16
I16 = mybir.dt.int16
AF = mybir.ActivationFunctionType
Alu = mybir.AluOpType
BF16NP = ml_dtypes.bfloat16

B, S, D = 2, 2048, 1024
H, DK, DV = 16, 64, 64
HL = 4          # heads per core
NPAIR = 2
ST = S // 128   # 16
DT = D // 128   # 8
SC = 512        # attention s-chunk
NSC = S // SC   # 4
N_CORES = 8

# int16(score*FEXP_A + FEXP_B) bit pattern read as bf16 ~= exp(score/8)
FEXP_A = 0.125 * 128.0 * 1.4426950408889634
FEXP_B = 16249.64
ACT_GROUPS = (0, 2, 4, 6)   # exact Act exp; remaining groups use DVE fastexp


class _TileContextSplitDrain(tile.TileContext):
    """Walrus in this container rejects ANY instruction carrying >1 sem wait
    ("Too many sync wait commands"). Post-lowering, sweep every basic block
    and move surplus waits onto injected EventSemaphore carrier instructions
    placed immediately before the over-subscribed instruction (same engine,
    same program point - semantics unchanged)."""

    _MAXW = 1

    def _split_excess_waits(self):
        nc = self.nc
        for fn in nc.m.functions:
            for bb in fn.blocks:
                insts = bb.instructions
                new_list = []
                changed = False
                for ins in insts:
                    si = ins.sync_info
                    waits = list(si.on_wait) if si is not None and si.on_wait else []
                    if len(waits) > self._MAXW:
                        changed = True
                        extra, keep = waits[:-self._MAXW], waits[-self._MAXW:]
                        for k in range(0, len(extra), self._MAXW):
                            chunk = extra[k:k + self._MAXW]
                            ev = mybir.InstEventSemaphore(
                                name=f"wsplit_{nc.next_id()}", ins=[], outs=[]
                            )
                            ev.engine = ins.engine
                            ev.sync_info = bass_rust.SyncInfo(
                                on_wait=chunk, on_update=[]
                            )
                            nc.register_instruction(ev, overwrite=True)
                            new_list.append(ev)
                        ins.sync_info = bass_rust.SyncInfo(
                            on_wait=keep,
                            on_update=list(si.on_update) if si.on_update else [],
                        )
                    new_list.append(ins)
                if changed:
                    insts[:] = new_list

    def _drain_and_barrier(self, tick_clock, wait_clock):
        self._split_excess_waits()
        ticks = list(tick_clock.global_clock)
        for p, t in enumerate(ticks):
            if t <= 0:
                continue
            v = bass_rust.VectorClock()
            v.require_at_least(p, t)
            d = self.nc.sync.drain()
            wait_clock.add_sem_waits(d.ins, ScopedClock({None: v}))
        self.nc.all_engine_barrier()
        popped = self.nc._tile_sem_poison_stack.pop()
        assert popped is self._sem_poison
        self.nc.clear_and_free_semaphores(list(self.sems.allocated().values()))
        self.nc.all_engine_barrier()


def build_nc() -> bass.Bass:
    nc = bass.Bass()

    # all inputs partition-major: row p holds everything partition p needs,
    # contiguously, so each dma_start is 128 large descriptors
    xT_d = nc.dram_tensor("xT", [128, NSC * DT * 512], BF16, kind="ExternalInput")
    wqk_d = nc.dram_tensor("wqk", [128, 4 * DT * 128], BF16, kind="ExternalInput")
    wv_d = nc.dram_tensor("wv", [128, DT * HL * DV], BF16, kind="ExternalInput")
    wo_d = nc.dram_tensor("wo", [128, NPAIR * D], BF16, kind="ExternalInput")
    bqk_d = nc.dram_tensor("bqk", [128, 4], F32, kind="ExternalInput")
    bv_d = nc.dram_tensor("bv", [1, HL * DV], F32, kind="ExternalInput")
    out_d = nc.dram_tensor("out", [S, D], F32, kind="ExternalOutput")

    with _TileContextSplitDrain(nc) as tc:
        with (
            tc.tile_pool(name="const", bufs=1) as constp,
            tc.tile_pool(name="persist", bufs=1) as pers,
        ):
            wqk_sb = constp.tile([128, DT, 4 * 128], BF16, tag="wqk")
            wv_sb = constp.tile([128, DT, HL * DV], BF16, tag="wv")
            wo_sb = constp.tile([128, NPAIR, D], BF16, tag="wo")
            bqk_sb = constp.tile([128, 4], F32, tag="bqk")
            bv_rep = constp.tile([128, HL * DV], F32, tag="bv_rep")
            xT_sb = pers.tile([128, DT, S], BF16, tag="xT")
            QT = pers.tile([128, NPAIR, S], BF16, tag="QT")
            KT = pers.tile([128, NPAIR, S], BF16, tag="KT")
            ctxT = pers.tile([128, NPAIR, S], BF16, tag="ctxT")
            V_aug = pers.tile([128, ST, HL, 128], BF16, tag="V_aug")

            def dma_xT(sc):
                nc.sync.dma_start(
                    out=xT_sb[:, :, sc * 512:(sc + 1) * 512],
                    in_=xT_d.rearrange("p (c d s) -> p c d s", c=NSC, d=DT)[:, sc, :, :],
                )

            wqk_r = wqk_d.rearrange("p (cb d c) -> p cb d c", cb=4, d=DT)
            dma_xT(0)
            nc.sync.dma_start(out=wqk_sb[:, :, 0:128], in_=wqk_r[:, 0, :, :])
            nc.sync.dma_start(out=wqk_sb[:, :, 128:256], in_=wqk_r[:, 1, :, :])
            dma_xT(1)
            nc.sync.dma_start(out=wqk_sb[:, :, 256:384], in_=wqk_r[:, 2, :, :])
            nc.sync.dma_start(out=wqk_sb[:, :, 384:512], in_=wqk_r[:, 3, :, :])
            nc.sync.dma_start(out=wv_sb, in_=wv_d[:, :])
            nc.sync.dma_start(out=bqk_sb, in_=bqk_d[:, :])
            nc.sync.dma_start(out=bv_rep, in_=bv_d[0:1, :].to_broadcast((128, HL * DV)))
            nc.vector.memset(V_aug[:, :, :, 64:128], 1.0)

            # ---------------- Phase 1: QKV projections ----------------------
            with (
                tc.tile_pool(name="qkps", bufs=5, space="PSUM") as qkp,
                tc.tile_pool(name="vps", bufs=3, space="PSUM") as vp,
            ):
                for sc in range(4):
                    if sc + 2 < 4:
                        dma_xT(sc + 2)
                    if sc == 1:
                        nc.sync.dma_start(out=wo_sb, in_=wo_d[:, :])
                    for proj, dest in ((0, QT), (1, KT)):
                        for pair in range(NPAIR):
                            col = (2 * proj + pair) * 128
                            ps = qkp.tile([128, 512], F32, tag="qkps")
                            for d in range(DT):
                                nc.tensor.matmul(
                                    ps[:, :],
                                    wqk_sb[:, d, col:col + 128],
                                    xT_sb[:, d, sc * 512:(sc + 1) * 512],
                                    start=(d == 0),
                                    stop=(d == DT - 1),
                                )
                            dst = dest[:, pair, sc * 512:(sc + 1) * 512]
                            bias = bqk_sb[:, 2 * proj + pair:2 * proj + pair + 1]
                            if pair == 0:
                                nc.scalar.activation(
                                    out=dst, in_=ps[:, :], func=AF.Identity,
                                    bias=bias, scale=1.0,
                                )
                            else:
                                nc.vector.tensor_scalar_add(dst, ps[:, :], bias)
                    for tt in range(4):
                        j = 4 * sc + tt
                        vps = vp.tile([128, HL * DV], F32, tag="vps")
                        for d in range(DT):
                            nc.tensor.matmul(
                                vps[:, :],
                                xT_sb[:, d, j * 128:(j + 1) * 128],
                                wv_sb[:, d, :],
                                start=(d == 0),
                                stop=(d == DT - 1),
                            )
                        nc.vector.tensor_add(
                            V_aug[:, j, :, 0:64],
                            vps.rearrange("p (h v) -> p h v", h=HL),
                            bv_rep.rearrange("p (h v) -> p h v", h=HL),
                        )

            # ---------------- Phase 2: attention with interleaved outproj ---
            with (
                tc.tile_pool(name="sps", bufs=2, space="PSUM") as spp,
                tc.tile_pool(name="cpp", bufs=2, space="PSUM") as cpp,
                tc.tile_pool(name="opp", bufs=2, space="PSUM") as opp,
                tc.tile_pool(name="ptp", bufs=3) as ptp,
                tc.tile_pool(name="recl", bufs=2) as recl,
                tc.tile_pool(name="recp", bufs=2) as recp,
                tc.tile_pool(name="otp", bufs=3) as otp,
            ):
                pending = []   # outproj steps of the previous chunk
                ot_ref = [None]

                def outproj_step(st, dc):
                    if dc == 0:
                        ot_ref[0] = otp.tile([128, D], F32, name="ot", tag="ot")
                    ot = ot_ref[0]
                    ops_t = opp.tile([128, 512], F32, name="ops", tag="ops")
                    for pair in range(NPAIR):
                        nc.tensor.matmul(
                            ops_t[:, :],
                            ctxT[:, pair, st * 128:(st + 1) * 128],
                            wo_sb[:, pair, dc * 512:(dc + 1) * 512],
                            start=(pair == 0),
                            stop=(pair == NPAIR - 1),
                        )
                    if dc == 0:
                        nc.scalar.copy(ot[:, 0:512], ops_t[:, :])
                    else:
                        nc.vector.tensor_copy(ot[:, 512:1024], ops_t[:, :])
                        nc.sync.dma_start(
                            out=out_d[st * 128:(st + 1) * 128, :], in_=ot
                        )

                for c in range(NSC):
                    for pair in range(NPAIR):
                        for e in range(2):
                            h = 2 * pair + e
                            base = 64 * e
                            cp = cpp.tile([128, SC], F32, name="cp", tag="cp")
                            sps = {}

                            def emit_scores(g, _pair=pair, _base=base, _c=c):
                                sp = spp.tile([128, 2, SC], F32, name="sp", tag="sp")
                                for k in range(2):
                                    j = 2 * g + k
                                    nc.tensor.matmul(
                                        sp[:, k, :],
                                        KT[_base:_base + 64, _pair, j * 128:(j + 1) * 128],
                                        QT[_base:_base + 64, _pair, _c * SC:(_c + 1) * SC],
                                        start=True,
                                        stop=True,
                                    )
                                sps[g] = sp

                            emit_scores(0)
                            emit_scores(1)
                            for g in range(8):
                                if g + 2 < 8:
                                    emit_scores(g + 2)
                                sp = sps.pop(g)
                                pt = ptp.tile([128, 2, SC], BF16, tag="pt")
                                if g in ACT_GROUPS:
                                    nc.scalar.activation(
                                        out=pt[:, :, :], in_=sp[:, :, :],
                                        func=AF.Exp, scale=0.125,
                                    )
                                else:
                                    nc.vector.tensor_scalar(
                                        pt.bitcast(I16), sp[:, :, :],
                                        FEXP_A, FEXP_B, Alu.mult, Alu.add,
                                    )
                                for k in range(2):
                                    j = 2 * g + k
                                    nc.tensor.matmul(
                                        cp[:, :],
                                        V_aug[:, j, h, :],
                                        pt[:, k, :],
                                        start=(g == 0 and k == 0),
                                        stop=(g == 7 and k == 1),
                                    )
                                if g % 2 == 1 and pending:
                                    pending.pop(0)()
                            # 1/den = exp(-ln(den)); den sits on psum rows
                            # 64..127 via the ones-columns of V_aug
                            rl = recl.tile([64, SC], F32, tag="rl")
                            nc.scalar.activation(out=rl, in_=cp[64:128, :], func=AF.Ln)
                            rec = recp.tile([64, SC], F32, tag="rec")
                            nc.scalar.activation(out=rec, in_=rl, func=AF.Exp, scale=-1.0)
                            nc.vector.tensor_mul(
                                ctxT[base:base + 64, pair, c * SC:(c + 1) * SC],
                                cp[0:64, :],
                                rec,
                            )
                    for st in range(4 * c, 4 * c + 4):
                        for dc in range(2):
                            pending.append(
                                lambda _st=st, _dc=dc: outproj_step(_st, _dc)
                            )
                while pending:
                    pending.pop(0)()

    return nc


_NC_CACHE = None


def get_nc() -> bass.Bass:
    global _NC_CACHE
    if _NC_CACHE is None:
        _NC_CACHE = build_nc()
    return _NC_CACHE


def prep_in_maps(hidden_state, Wq, bq, Wk, bk, Wv, bv, Wo, bo):
    hidden_state = np.asarray(hidden_state, np.float32)
    Wq, bq = np.asarray(Wq, np.float32), np.asarray(bq, np.float32)
    Wk, bk = np.asarray(Wk, np.float32), np.asarray(bk, np.float32)
    Wv, bv = np.asarray(Wv, np.float32), np.asarray(bv, np.float32)
    Wo, bo = np.asarray(Wo, np.float32), np.asarray(bo, np.float32)

    in_maps = []
    for core in range(N_CORES):
        b, g = core // 4, core % 4
        hs = slice(HL * g, HL * (g + 1))
        # [D, C] -> [128, DT, C]: row p holds d-tiles d*128+p contiguously
        def pmaj(a):
            return np.ascontiguousarray(
                a.reshape(DT, 128, -1).transpose(1, 0, 2)
            ).reshape(128, -1)

        xT = hidden_state[b].T.astype(BF16NP)
        # [128, NSC, DT, 512]
        xT = np.ascontiguousarray(
            xT.reshape(DT, 128, NSC, 512).transpose(1, 2, 0, 3)
        ).reshape(128, -1)
        cols = []
        for wmat in (Wq[hs], Wk[hs]):
            for pair in range(NPAIR):
                cols.append(
                    wmat[2 * pair:2 * pair + 2].transpose(1, 0, 2).reshape(D, 128)
                )
        # [D, 4*128] -> [128, colblk, DT, 128] partition-major per block
        wqk = np.ascontiguousarray(
            np.concatenate(cols, axis=1).astype(BF16NP)
            .reshape(DT, 128, 4, 128).transpose(1, 2, 0, 3)
        ).reshape(128, -1)
        wv_g = pmaj(Wv[hs].transpose(1, 0, 2).reshape(D, HL * DV).astype(BF16NP))
        wo_g = np.ascontiguousarray(
            Wo[HL * DV * g: HL * DV * (g + 1)].astype(BF16NP)
            .reshape(NPAIR, 128, D).transpose(1, 0, 2)
        ).reshape(128, -1)
        bqk_cols = []
        for bvec in (bq[hs], bk[hs]):
            for pair in range(NPAIR):
                bqk_cols.append(bvec[2 * pair:2 * pair + 2].reshape(128))
        bqk = np.stack(bqk_cols, axis=1).astype(np.float32)
        in_maps.append({
            "xT": xT,
            "wqk": np.ascontiguousarray(wqk),
            "wv": np.ascontiguousarray(wv_g),
            "wo": wo_g,
            "bqk": np.ascontiguousarray(bqk),
            "bv": np.ascontiguousarray(bv[hs].reshape(1, HL * DV)),
        })
    return in_maps


_BO = None


def gather(results):
    """Sum the 4 row-parallel partials per batch, then add bo."""
    out = np.empty((B, S, D), np.float32)
    for b in range(B):
        acc = results[4 * b]["out"].astype(np.float32)
        for g in range(1, 4):
            acc = acc + results[4 * b + g]["out"]
        out[b] = acc + _BO[None, :]
    return out


def kernel(**inputs) -> np.ndarray:
    global _BO
    _BO = np.asarray(inputs["bo"], np.float32)
    nc = get_nc()
    in_maps = prep_in_maps(**inputs)
    res = run_bass_kernel_spmd(nc, in_maps, core_ids=list(range(N_CORES)))
    return gather(res.results)
